# revision 4
# baseline (speedup 1.0000x reference)
"""Trainium2 Bass kernel for nn_JointsLoss_sinkhorn.

The reference computes sinkhorn iterations whose outputs are discarded; the
returned loss is just

    loss = 0.5 * mean(((output - target) * target_weight) ** 2)

over output/target of shape [B=256, J=17, H=64, W=48] (weight broadcast over
the HW axis). That is a pure memory-bound weighted-MSE reduction over ~107 MB.

Sharding: pure data parallel over the batch dim — each of the 8 cores reduces
its 32-batch shard to [128, 5] weighted row sums; the host adds them up and
applies the 0.5/count scale.

Device layout: rows = (b, j) pairs -> 544 rows of 3072 per core. The host
interleaves output/target row-wise into one [544, 2, 3072] tensor so each
row-tile is ONE contiguous ~3 MB DMA. Per row-tile of <=128 rows:
    DVE  tensor_sub   d = o - t                       (in place)
    ACT  activation   Square(w[p] * d) + accum_out -> sum_f w^2 (o-t)^2
Raw bass (explicit semaphores, standalone waits): this container's walrus
rejects instructions carrying multi-wait sync_info, so Tile-emitted programs
don't compile; standalone wait_ge + then_inc do.
"""

from contextlib import ExitStack

import numpy as np

import concourse.bass as bass
from concourse import mybir
from concourse.bass_utils import run_bass_kernel_spmd

B, J, H, W = 256, 17, 64, 48
HW = H * W                      # 3072
N_CORES = 8
B_SHARD = B // N_CORES          # 32
ROWS = B_SHARD * J              # 544 (b,j) rows per core
P = 128
N_TILES = -(-ROWS // P)         # 5: 4 full tiles + one 32-row tail
NBUF = 3

_CACHE = {}

# Results of the most recent run (BassKernelResults) — exposed for test
# harnesses that want exec_time_ns / trace paths.
LAST_RESULTS = None


def _build_nc():
    nc = bass.Bass()
    ot = nc.dram_tensor("ot", [ROWS, 2, HW], mybir.dt.float32, kind="ExternalInput")
    # Per-row weights, host-padded to [P, N_TILES]: w[p, i] = weight of row
    # i*128+p (zero beyond row 543).
    w = nc.dram_tensor("w", [P, N_TILES], mybir.dt.float32, kind="ExternalInput")
    res = nc.dram_tensor("res", [P, N_TILES], mybir.dt.float32, kind="ExternalOutput")

    with ExitStack() as ctx:
        bufs = [
            ctx.enter_context(
                nc.sbuf_tensor(f"ot{k}", [P, 2, HW], mybir.dt.float32)
            )
            for k in range(NBUF)
        ]
        w_sb = ctx.enter_context(nc.sbuf_tensor("w_sb", [P, N_TILES], mybir.dt.float32))
        s_sb = ctx.enter_context(nc.sbuf_tensor("s_sb", [P, N_TILES], mybir.dt.float32))
        w_sem = ctx.enter_context(nc.semaphore("w_sem"))
        t_sems = [ctx.enter_context(nc.semaphore(f"t_sem{i}")) for i in range(N_TILES)]
        dve_sem = ctx.enter_context(nc.semaphore("dve_sem"))
        act_sem = ctx.enter_context(nc.semaphore("act_sem"))
        o_sem = ctx.enter_context(nc.semaphore("o_sem"))
        block = ctx.enter_context(nc.Block())

        @block.sync
        def _(sync):
            sync.dma_start(out=w_sb[:], in_=w[:, :]).then_inc(w_sem, 16)
            for i in range(N_TILES):
                r = min(P, ROWS - i * P)
                if i >= NBUF:
                    # slot free once the ACT that read it is done
                    sync.wait_ge(act_sem, i - NBUF + 1)
                sync.dma_start(
                    out=bufs[i % NBUF][:r], in_=ot[i * P : i * P + r, :, :]
                ).then_inc(t_sems[i], 16)
            sync.wait_ge(act_sem, N_TILES)
            sync.dma_start(out=res[:, :], in_=s_sb[:]).then_inc(o_sem, 16)
            sync.wait_ge(o_sem, 16)

        @block.vector
        def _(vector):
            vector.memset(s_sb[:], 0.0).then_inc(dve_sem, 1)
            for i in range(N_TILES):
                r = min(P, ROWS - i * P)
                buf = bufs[i % NBUF]
                vector.wait_ge(t_sems[i], 16)
                vector.tensor_sub(
                    out=buf[:r, 0, :], in0=buf[:r, 0, :], in1=buf[:r, 1, :]
                ).then_inc(dve_sem, 1)

        @block.scalar
        def _(scalar):
            scalar.wait_ge(w_sem, 16)
            for i in range(N_TILES):
                r = min(P, ROWS - i * P)
                buf = bufs[i % NBUF]
                scalar.wait_ge(dve_sem, 2 + i)
                scalar.activation(
                    out=buf[:r, 0, :],
                    in_=buf[:r, 0, :],
                    func=mybir.ActivationFunctionType.Square,
                    scale=w_sb[:r, i : i + 1],
                    accum_out=s_sb[:r, i : i + 1],
                ).then_inc(act_sem, 1)

    return nc


def _get_nc():
    if "nc" not in _CACHE:
        _CACHE["nc"] = _build_nc()
    return _CACHE["nc"]


def _make_in_maps(output, target, target_weight):
    o_flat = np.ascontiguousarray(output, dtype=np.float32).reshape(B * J, HW)
    t_flat = np.ascontiguousarray(target, dtype=np.float32).reshape(B * J, HW)
    w_flat = np.ascontiguousarray(target_weight, dtype=np.float32).reshape(B * J)
    in_maps = []
    for c in range(N_CORES):
        rows = slice(c * ROWS, (c + 1) * ROWS)
        ot = np.empty((ROWS, 2, HW), np.float32)
        ot[:, 0, :] = o_flat[rows]
        ot[:, 1, :] = t_flat[rows]
        w_pad = np.zeros(P * N_TILES, np.float32)
        w_pad[:ROWS] = w_flat[rows]
        in_maps.append(
            {
                "ot": ot,
                "w": np.ascontiguousarray(w_pad.reshape(N_TILES, P).T),
            }
        )
    return in_maps


def kernel(output, target, target_weight, _trace=False):
    global LAST_RESULTS
    nc = _get_nc()
    in_maps = _make_in_maps(output, target, target_weight)
    results = run_bass_kernel_spmd(
        nc, in_maps, core_ids=list(range(N_CORES)), trace=_trace
    )
    LAST_RESULTS = results
    total = sum(float(r["res"].sum(dtype=np.float64)) for r in results.results)
    return np.asarray(0.5 * total / (B * J * HW), dtype=np.float32)
